# revision 2
# baseline (speedup 1.0000x reference)
"""LowRankProcessNeurons Trainium2 kernel: 8-core data-parallel over tokens.

Math: gates = sigmoid(conv1d(enriched) + b); per-expert low-rank transform
collapses to two matmuls with the expert sum fused into PSUM accumulation:
  D^T[n] = W_down[n]^T @ X^T          (16 experts, K=H accumulate)
  Ds^T[n] = D^T[n] * gates^T[n]       (DVE, gate broadcast via DMA)
  Y = sum_n Ds[n] @ W_up[n]           (PSUM accumulate over experts)
"""
import numpy as np

import concourse.bass as bass
import concourse.mybir as mybir
import concourse.tile as tile
from concourse import bass_utils
from concourse.masks import make_identity

F32 = mybir.dt.float32
F32R = mybir.dt.float32r

B, S, H, N_IN, N_PROC, R = 4, 1024, 1024, 64, 16, 128
N_CORES = 8
T = (B * S) // N_CORES          # 512 tokens per core
KH = H // 128                   # 8 contraction chunks for down matmul
TM = T // 128                   # 4 token tiles
ET_W = T + 4                    # halo'd enriched width


def _split_multiwait_drains(nc):
    # this walrus build rejects >1 sync wait per instruction; hoist all but
    # one wait onto preceding single-wait NoOps on the same engine.
    for f in nc.m.functions:
        for blk in f.blocks:
            insts = blk.instructions
            i = 0
            while i < len(insts):
                inst = insts[i]
                si = inst.sync_info
                if si is not None and si.on_wait and len(si.on_wait) > 1:
                    waits = list(si.on_wait)
                    si.on_wait = waits[-1:]
                    new = []
                    for w in waits[:-1]:
                        nd = mybir.InstNoOp(
                            name=nc.get_next_instruction_name(),
                            engine=inst.engine, ins=[], outs=[],
                            sync_info=mybir.SyncInfo(on_wait=[w], on_update=[]))
                        nc.register_instruction(nd)
                        new.append(nd)
                    insts[i:i] = new
                    i += len(new)
                i += 1


def _build():
    nc = bass.Bass(trn_type="TRN2")
    x = nc.dram_tensor("x", [T, H], F32, kind="ExternalInput")
    et = nc.dram_tensor("et", [N_IN, ET_W], F32R, kind="ExternalInput")
    wc = nc.dram_tensor("wc", [5, N_IN, N_PROC], F32R, kind="ExternalInput")
    cb = nc.dram_tensor("cb", [N_PROC, 1], F32, kind="ExternalInput")
    wd = nc.dram_tensor("wd", [N_PROC, H, R], F32R, kind="ExternalInput")
    wu = nc.dram_tensor("wu", [N_PROC, R, H], F32R, kind="ExternalInput")
    y = nc.dram_tensor("y", [T, H], F32, kind="ExternalOutput")
    g = nc.dram_tensor("g", [N_PROC, T], F32, kind="ExternalOutput")

    with tile.TileContext(nc) as tc:
        with tc.tile_pool(name="consts", bufs=1) as consts, \
             tc.tile_pool(name="weights", bufs=1) as wpool, \
             tc.tile_pool(name="dram", bufs=1, space="DRAM") as dpool:

            ident = consts.tile([128, 128], F32)
            make_identity(nc, ident)
            t_et = consts.tile([N_IN, ET_W], F32R)
            t_wc = consts.tile([N_IN, 5, N_PROC], F32R)
            t_cb = consts.tile([N_PROC, 1], F32)
            # conv inputs on ACT queue (ahead of gbc/wu there)
            nc.scalar.dma_start(out=t_et, in_=et.ap())
            for k in range(5):
                nc.scalar.dma_start(out=t_wc[:, k, :], in_=wc.ap()[k])
            nc.scalar.dma_start(out=t_cb, in_=cb.ap())

            # big resident weights
            t_wd = wpool.tile([128, N_PROC, KH, 128], F32R)   # [h, n, k, r]
            t_wu = wpool.tile([128, N_PROC, H], F32R)         # [r, n, h]
            t_xt = wpool.tile([128, KH, T], F32R)             # X^T chunks
            for n in range(N_PROC):
                for k in range(KH):
                    nc.sync.dma_start(out=t_wd[:, n, k, :],
                                      in_=wd.ap()[n, k * 128:(k + 1) * 128, :])
            for n in range(N_PROC):
                nc.scalar.dma_start(out=t_wu[:, n, :], in_=wu.ap()[n])

            d_g = dpool.tile([N_PROC, T], F32)   # gates roundtrip for bcast

            with tc.tile_pool(name="xin", bufs=4) as xin, \
                 tc.tile_pool(name="ptr", bufs=2, space="PSUM") as ptr, \
                 tc.tile_pool(name="pconv", bufs=1, space="PSUM") as pconv:

                # ---- conv gates: pr^T[16, T] = sum_k Wc_k^T @ E^T[:, k:k+T]
                p_cv = pconv.tile([N_PROC, T], F32)
                for k in range(5):
                    nc.tensor.matmul(p_cv, t_wc[:, k, :], t_et[:, k:k + T],
                                     start=(k == 0), stop=(k == 4))
                g_final = consts.tile([N_PROC, T], F32)
                nc.scalar.activation(out=g_final, in_=p_cv,
                                     func=mybir.ActivationFunctionType.Sigmoid,
                                     bias=t_cb, scale=1.0)
                nc.sync.dma_start(out=g.ap(), in_=g_final)
                nc.scalar.dma_start(out=d_g, in_=g_final)

                # ---- X^T via PE transpose, per k-chunk collect 4 m-blocks
                t_x = []
                for m in range(TM):
                    xm = xin.tile([128, H], F32, tag="xm")
                    nc.sync.dma_start(out=xm, in_=x.ap()[m * 128:(m + 1) * 128, :])
                    t_x.append(xm)
                for k in range(KH):
                    p_t = ptr.tile([128, TM, 128], F32, tag="pt")
                    for m in range(TM):
                        nc.tensor.transpose(p_t[:, m, :],
                                            t_x[m][:, k * 128:(k + 1) * 128], ident)
                    nc.vector.tensor_copy(t_xt[:, k, :],
                                          p_t.rearrange("p m t -> p (m t)"))

            with tc.tile_pool(name="dsT", bufs=1) as dspool, \
                 tc.tile_pool(name="gbc", bufs=5) as gbcp, \
                 tc.tile_pool(name="ysb", bufs=3) as ysb, \
                 tc.tile_pool(name="pd", bufs=3, space="PSUM") as pd, \
                 tc.tile_pool(name="py", bufs=3, space="PSUM") as py:

                t_ds = dspool.tile([128, N_PROC, T], F32R)    # Ds^T [r, n, t]

                # ---- down matmuls + gate scale per expert
                for n in range(N_PROC):
                    gb = gbcp.tile([128, T], F32, tag="gb")
                    nc.scalar.dma_start(
                        out=gb, in_=d_g[n:n + 1, :].partition_broadcast(128))
                    p_d = pd.tile([128, T], F32, tag="pd")
                    for k in range(KH):
                        nc.tensor.matmul(p_d, t_wd[:, n, k, :], t_xt[:, k, :],
                                         start=(k == 0), stop=(k == KH - 1))
                    nc.vector.tensor_mul(t_ds[:, n, :], p_d, gb)

                # ---- up matmuls: Y[m-tile, h-half] = sum_n Ds^T[n]^T @ Wu[n]
                for m in range(TM):
                    for h2 in range(2):
                        p_y = py.tile([128, 512], F32, tag="py")
                        for n in range(N_PROC):
                            nc.tensor.matmul(
                                p_y,
                                t_ds[:, n, m * 128:(m + 1) * 128],
                                t_wu[:, n, h2 * 512:(h2 + 1) * 512],
                                start=(n == 0), stop=(n == N_PROC - 1))
                        ys = ysb.tile([128, 512], F32, tag="ys")
                        nc.vector.tensor_copy(ys, p_y)
                        nc.sync.dma_start(
                            out=y.ap()[m * 128:(m + 1) * 128,
                                       h2 * 512:(h2 + 1) * 512],
                            in_=ys)
    _split_multiwait_drains(nc)
    return nc


_NC_CACHE = None


def _get_nc():
    global _NC_CACHE
    if _NC_CACHE is None:
        _NC_CACHE = _build()
    return _NC_CACHE


def _prep_in_maps(intermediate, enriched_activations, conv_w, conv_b,
                  down_proj, up_proj):
    X2 = np.ascontiguousarray(intermediate.reshape(B * S, H), dtype=np.float32)
    E2 = np.asarray(enriched_activations.reshape(B * S, N_IN), dtype=np.float32)
    wc_h = np.ascontiguousarray(np.asarray(conv_w, np.float32)[:, 0].transpose(1, 2, 0))
    cb_h = np.ascontiguousarray(np.asarray(conv_b, np.float32).reshape(N_PROC, 1))
    wd_h = np.ascontiguousarray(np.asarray(down_proj, np.float32))
    wu_h = np.ascontiguousarray(np.asarray(up_proj, np.float32))
    in_maps = []
    for c in range(N_CORES):
        s0 = c * T
        b = s0 // S
        ep = np.zeros((ET_W, N_IN), np.float32)
        lo, hi = s0 - 2, s0 + T + 2
        vlo, vhi = max(lo, b * S), min(hi, (b + 1) * S)
        ep[vlo - lo:vhi - lo] = E2[vlo:vhi]
        in_maps.append({
            "x": X2[s0:s0 + T].copy(),
            "et": np.ascontiguousarray(ep.T),
            "wc": wc_h, "cb": cb_h, "wd": wd_h, "wu": wu_h,
        })
    return in_maps


def run(inputs, trace=False, trace_kwargs=None):
    nc = _get_nc()
    in_maps = _prep_in_maps(**inputs)
    res = bass_utils.run_bass_kernel_spmd(
        nc, in_maps, core_ids=list(range(N_CORES)), trace=trace,
        **(trace_kwargs or {}))
    ys = np.concatenate([res.results[c]["y"] for c in range(N_CORES)], axis=0)
    gs = np.concatenate([res.results[c]["g"].T for c in range(N_CORES)], axis=0)
    out = ys.reshape(B, S, H).astype(np.float32)
    gates = gs.reshape(B, S, N_PROC).astype(np.float32)
    return (out, gates), res


def kernel(**inputs):
    outs, _ = run(inputs, trace=False)
    return outs


# revision 3
# speedup vs baseline: 1.7014x; 1.7014x over previous
"""LowRankProcessNeurons Trainium2 kernel: 8-core data-parallel over tokens.

Math: gates = sigmoid(conv1d(enriched) + b); per-expert low-rank transform
collapses to two matmuls with the expert sum fused into PSUM accumulation:
  D^T[n] = W_down[n]^T @ X^T          (16 experts, K=H accumulate)
  Ds^T[n] = D^T[n] * gates^T[n]       (DVE, gate broadcast via DMA)
  Y = sum_n Ds[n] @ W_up[n]           (PSUM accumulate over experts)
Weights are host-packed into SBUF partition layout and cast to bf16 so the
weight streams are two contiguous DMAs instead of strided 512B bursts.
"""
import numpy as np
import ml_dtypes

import concourse.bass as bass
import concourse.mybir as mybir
import concourse.tile as tile
from concourse import bass_utils
from concourse.masks import make_identity

F32 = mybir.dt.float32
F32R = mybir.dt.float32r
BF16 = mybir.dt.bfloat16

B, S, H, N_IN, N_PROC, R = 4, 1024, 1024, 64, 16, 128
N_CORES = 8
T = (B * S) // N_CORES          # 512 tokens per core
KH = H // 128                   # 8 contraction chunks for down matmul
TM = T // 128                   # 4 token tiles
ET_W = T + 4                    # halo'd enriched width
NG = 4                          # weight DMA chunks (experts per chunk)


def _split_multiwait_drains(nc):
    # this walrus build rejects >1 sync wait per instruction; hoist all but
    # one wait onto preceding single-wait NoOps on the same engine.
    for f in nc.m.functions:
        for blk in f.blocks:
            insts = blk.instructions
            i = 0
            while i < len(insts):
                inst = insts[i]
                si = inst.sync_info
                if si is not None and si.on_wait and len(si.on_wait) > 1:
                    waits = list(si.on_wait)
                    si.on_wait = waits[-1:]
                    new = []
                    for w in waits[:-1]:
                        nd = mybir.InstNoOp(
                            name=nc.get_next_instruction_name(),
                            engine=inst.engine, ins=[], outs=[],
                            sync_info=mybir.SyncInfo(on_wait=[w], on_update=[]))
                        nc.register_instruction(nd)
                        new.append(nd)
                    insts[i:i] = new
                    i += len(new)
                i += 1


def _build():
    nc = bass.Bass(trn_type="TRN2")
    x = nc.dram_tensor("x", [T, H], F32, kind="ExternalInput")
    et = nc.dram_tensor("et", [N_IN, ET_W], F32R, kind="ExternalInput")
    wc = nc.dram_tensor("wc", [5, N_IN, N_PROC], F32R, kind="ExternalInput")
    cb = nc.dram_tensor("cb", [N_PROC, 1], F32, kind="ExternalInput")
    # host-packed SBUF layouts, bf16
    wd = nc.dram_tensor("wd", [128, N_PROC, KH, 128], BF16, kind="ExternalInput")
    wu = nc.dram_tensor("wu", [128, N_PROC, H], BF16, kind="ExternalInput")
    y = nc.dram_tensor("y", [T, H], F32, kind="ExternalOutput")
    g = nc.dram_tensor("g", [N_PROC, T], F32, kind="ExternalOutput")

    with tile.TileContext(nc) as tc:
        with tc.tile_pool(name="consts", bufs=1) as consts, \
             tc.tile_pool(name="weights", bufs=1) as wpool, \
             tc.tile_pool(name="dram", bufs=1, space="DRAM") as dpool:

            ident = consts.tile([128, 128], F32)
            make_identity(nc, ident)
            t_et = consts.tile([N_IN, ET_W], F32R)
            t_wc = consts.tile([N_IN, 5, N_PROC], F32R)
            t_cb = consts.tile([N_PROC, 1], F32)
            # conv inputs first on ACT queue (gates gate everything)
            nc.scalar.dma_start(out=t_et, in_=et.ap())
            for k in range(5):
                nc.scalar.dma_start(out=t_wc[:, k, :], in_=wc.ap()[k])
            nc.scalar.dma_start(out=t_cb, in_=cb.ap())

            # big resident weights: contiguous chunked DMAs
            t_wd = wpool.tile([128, N_PROC, KH, 128], BF16)   # [h, n, k, r]
            t_wu = wpool.tile([128, N_PROC, H], BF16)         # [r, n, h]
            t_xt = wpool.tile([128, KH, T], BF16)             # X^T chunks
            ne = N_PROC // NG
            for gi in range(NG):
                nc.sync.dma_start(out=t_wd[:, gi * ne:(gi + 1) * ne],
                                  in_=wd.ap()[:, gi * ne:(gi + 1) * ne])
            for gi in range(NG):
                nc.scalar.dma_start(out=t_wu[:, gi * ne:(gi + 1) * ne],
                                    in_=wu.ap()[:, gi * ne:(gi + 1) * ne])

            d_g = dpool.tile([N_PROC, T], BF16)   # gates roundtrip for bcast

            with tc.tile_pool(name="xin", bufs=4) as xin, \
                 tc.tile_pool(name="ptr", bufs=2, space="PSUM") as ptr, \
                 tc.tile_pool(name="pconv", bufs=1, space="PSUM") as pconv:

                # ---- conv gates: pr^T[16, T] = sum_k Wc_k^T @ E^T[:, k:k+T]
                p_cv = pconv.tile([N_PROC, T], F32)
                for k in range(5):
                    nc.tensor.matmul(p_cv, t_wc[:, k, :], t_et[:, k:k + T],
                                     start=(k == 0), stop=(k == 4))
                g_final = consts.tile([N_PROC, T], F32)
                nc.scalar.activation(out=g_final, in_=p_cv,
                                     func=mybir.ActivationFunctionType.Sigmoid,
                                     bias=t_cb, scale=1.0)
                nc.sync.dma_start(out=g.ap(), in_=g_final)
                g_bf = consts.tile([N_PROC, T], BF16)
                nc.scalar.copy(g_bf, g_final)
                nc.scalar.dma_start(out=d_g, in_=g_bf)

                # ---- X^T via PE transpose, per k-chunk collect 4 m-blocks
                t_x = []
                for m in range(TM):
                    xm = xin.tile([128, H], F32, tag="xm")
                    nc.sync.dma_start(out=xm, in_=x.ap()[m * 128:(m + 1) * 128, :])
                    t_x.append(xm)
                for k in range(KH):
                    p_t = ptr.tile([128, TM, 128], F32, tag="pt")
                    for m in range(TM):
                        nc.tensor.transpose(p_t[:, m, :],
                                            t_x[m][:, k * 128:(k + 1) * 128], ident)
                    nc.vector.tensor_copy(t_xt[:, k, :],
                                          p_t.rearrange("p m t -> p (m t)"))

            with tc.tile_pool(name="dsT", bufs=1) as dspool, \
                 tc.tile_pool(name="gbc", bufs=5) as gbcp, \
                 tc.tile_pool(name="ysb", bufs=3) as ysb, \
                 tc.tile_pool(name="pd", bufs=3, space="PSUM") as pd, \
                 tc.tile_pool(name="py", bufs=3, space="PSUM") as py:

                t_ds = dspool.tile([128, N_PROC, T], BF16)    # Ds^T [r, n, t]

                # ---- down matmuls + gate scale per expert
                for n in range(N_PROC):
                    gb = gbcp.tile([128, T], BF16, tag="gb")
                    nc.scalar.dma_start(
                        out=gb, in_=d_g[n:n + 1, :].partition_broadcast(128))
                    p_d = pd.tile([128, T], F32, tag="pd")
                    for k in range(KH):
                        nc.tensor.matmul(p_d, t_wd[:, n, k, :], t_xt[:, k, :],
                                         start=(k == 0), stop=(k == KH - 1))
                    nc.vector.tensor_mul(t_ds[:, n, :], p_d, gb)

                # ---- up matmuls: Y[m-tile, h-half] = sum_n Ds^T[n]^T @ Wu[n]
                for m in range(TM):
                    for h2 in range(2):
                        p_y = py.tile([128, 512], F32, tag="py")
                        for n in range(N_PROC):
                            nc.tensor.matmul(
                                p_y,
                                t_ds[:, n, m * 128:(m + 1) * 128],
                                t_wu[:, n, h2 * 512:(h2 + 1) * 512],
                                start=(n == 0), stop=(n == N_PROC - 1))
                        ys = ysb.tile([128, 512], F32, tag="ys")
                        nc.vector.tensor_copy(ys, p_y)
                        eng = nc.sync if (m < 2) else nc.scalar
                        eng.dma_start(
                            out=y.ap()[m * 128:(m + 1) * 128,
                                       h2 * 512:(h2 + 1) * 512],
                            in_=ys)
    _split_multiwait_drains(nc)
    return nc


_NC_CACHE = None


def _get_nc():
    global _NC_CACHE
    if _NC_CACHE is None:
        _NC_CACHE = _build()
    return _NC_CACHE


def _prep_in_maps(intermediate, enriched_activations, conv_w, conv_b,
                  down_proj, up_proj):
    X2 = np.ascontiguousarray(intermediate.reshape(B * S, H), dtype=np.float32)
    E2 = np.asarray(enriched_activations.reshape(B * S, N_IN), dtype=np.float32)
    wc_h = np.ascontiguousarray(np.asarray(conv_w, np.float32)[:, 0].transpose(1, 2, 0))
    cb_h = np.ascontiguousarray(np.asarray(conv_b, np.float32).reshape(N_PROC, 1))
    # pack weights into SBUF partition layout, bf16:
    # wd[p, n, k, r] = down_proj[n, k*128+p, r]
    wd_f = np.asarray(down_proj, np.float32).reshape(N_PROC, KH, 128, R)
    wd_h = np.ascontiguousarray(wd_f.transpose(2, 0, 1, 3).astype(ml_dtypes.bfloat16))
    # wu[p, n, h] = up_proj[n, p, h]
    wu_h = np.ascontiguousarray(
        np.asarray(up_proj, np.float32).transpose(1, 0, 2).astype(ml_dtypes.bfloat16))
    in_maps = []
    for c in range(N_CORES):
        s0 = c * T
        b = s0 // S
        ep = np.zeros((ET_W, N_IN), np.float32)
        lo, hi = s0 - 2, s0 + T + 2
        vlo, vhi = max(lo, b * S), min(hi, (b + 1) * S)
        ep[vlo - lo:vhi - lo] = E2[vlo:vhi]
        in_maps.append({
            "x": X2[s0:s0 + T].copy(),
            "et": np.ascontiguousarray(ep.T),
            "wc": wc_h, "cb": cb_h, "wd": wd_h, "wu": wu_h,
        })
    return in_maps


def run(inputs, trace=False, trace_kwargs=None):
    nc = _get_nc()
    in_maps = _prep_in_maps(**inputs)
    res = bass_utils.run_bass_kernel_spmd(
        nc, in_maps, core_ids=list(range(N_CORES)), trace=trace,
        **(trace_kwargs or {}))
    ys = np.concatenate([res.results[c]["y"] for c in range(N_CORES)], axis=0)
    gs = np.concatenate([res.results[c]["g"].T for c in range(N_CORES)], axis=0)
    out = ys.reshape(B, S, H).astype(np.float32)
    gates = gs.reshape(B, S, N_PROC).astype(np.float32)
    return (out, gates), res


def kernel(**inputs):
    outs, _ = run(inputs, trace=False)
    return outs


# revision 4
# speedup vs baseline: 1.8470x; 1.0856x over previous
"""LowRankProcessNeurons Trainium2 kernel: 8-core data-parallel over tokens.

Math: gates = sigmoid(conv1d(enriched) + b); per-expert low-rank transform
collapses to two matmuls with the expert sum fused into PSUM accumulation:
  D^T[n] = W_down[n]^T @ X^T          (16 experts, K=H accumulate)
  Ds^T[n] = D^T[n] * gates^T[n]       (DVE, gate broadcast via DMA)
  Y = sum_n Ds[n] @ W_up[n]           (PSUM accumulate over experts)
Weights are host-packed into SBUF partition layout and cast to bf16 so the
weight streams are two contiguous DMAs instead of strided 512B bursts.
"""
import numpy as np
import ml_dtypes

import concourse.bass as bass
import concourse.mybir as mybir
import concourse.tile as tile
from concourse import bass_utils
from concourse.masks import make_identity

F32 = mybir.dt.float32
F32R = mybir.dt.float32r
BF16 = mybir.dt.bfloat16

B, S, H, N_IN, N_PROC, R = 4, 1024, 1024, 64, 16, 128
N_CORES = 8
T = (B * S) // N_CORES          # 512 tokens per core
KH = H // 128                   # 8 contraction chunks for down matmul
TM = T // 128                   # 4 token tiles
ET_W = T + 4                    # halo'd enriched width
NG = 4                          # weight DMA chunks (experts per chunk)


def _split_multiwait_drains(nc):
    # this walrus build rejects >1 sync wait per instruction; hoist all but
    # one wait onto preceding single-wait NoOps on the same engine.
    for f in nc.m.functions:
        for blk in f.blocks:
            insts = blk.instructions
            i = 0
            while i < len(insts):
                inst = insts[i]
                si = inst.sync_info
                if si is not None and si.on_wait and len(si.on_wait) > 1:
                    waits = list(si.on_wait)
                    si.on_wait = waits[-1:]
                    new = []
                    for w in waits[:-1]:
                        nd = mybir.InstNoOp(
                            name=nc.get_next_instruction_name(),
                            engine=inst.engine, ins=[], outs=[],
                            sync_info=mybir.SyncInfo(on_wait=[w], on_update=[]))
                        nc.register_instruction(nd)
                        new.append(nd)
                    insts[i:i] = new
                    i += len(new)
                i += 1


def _build():
    nc = bass.Bass(trn_type="TRN2")
    x = nc.dram_tensor("x", [T, H], F32, kind="ExternalInput")
    et = nc.dram_tensor("et", [N_IN, ET_W], F32R, kind="ExternalInput")
    wc = nc.dram_tensor("wc", [5, N_IN, N_PROC], F32R, kind="ExternalInput")
    cb = nc.dram_tensor("cb", [N_PROC, 1], F32, kind="ExternalInput")
    # host-packed SBUF layouts, bf16
    wd = nc.dram_tensor("wd", [128, N_PROC, KH, 128], BF16, kind="ExternalInput")
    wu = nc.dram_tensor("wu", [128, N_PROC, H], BF16, kind="ExternalInput")
    y = nc.dram_tensor("y", [T, H], F32, kind="ExternalOutput")
    g = nc.dram_tensor("g", [N_PROC, T], F32, kind="ExternalOutput")

    with tile.TileContext(nc) as tc:
        with tc.tile_pool(name="consts", bufs=1) as consts, \
             tc.tile_pool(name="weights", bufs=1) as wpool, \
             tc.tile_pool(name="dram", bufs=1, space="DRAM") as dpool:

            ident = consts.tile([128, 128], F32)
            make_identity(nc, ident)
            t_et = consts.tile([N_IN, ET_W], F32R)
            t_wc = consts.tile([N_IN, 5, N_PROC], F32R)
            t_cb = consts.tile([N_PROC, 1], F32)
            # conv inputs first on ACT queue (gates gate everything)
            nc.scalar.dma_start(out=t_et, in_=et.ap())
            for k in range(5):
                nc.scalar.dma_start(out=t_wc[:, k, :], in_=wc.ap()[k])
            nc.scalar.dma_start(out=t_cb, in_=cb.ap())

            # big resident weights: contiguous chunked DMAs
            t_wd = wpool.tile([128, N_PROC, KH, 128], BF16)   # [h, n, k, r]
            t_wu = wpool.tile([128, N_PROC, H], BF16)         # [r, n, h]
            t_xt = wpool.tile([128, KH, T], BF16)             # X^T chunks
            ne = N_PROC // NG

            d_g = dpool.tile([N_PROC, T], BF16)   # gates roundtrip for bcast

            with tc.tile_pool(name="xin", bufs=4) as xin, \
                 tc.tile_pool(name="ptr", bufs=2, space="PSUM") as ptr, \
                 tc.tile_pool(name="pconv", bufs=1, space="PSUM") as pconv:

                # ---- x tiles first on SP (transposes need them early)
                t_x = []
                for m in range(TM):
                    xm = xin.tile([128, H], F32, tag="xm")
                    nc.sync.dma_start(out=xm, in_=x.ap()[m * 128:(m + 1) * 128, :])
                    t_x.append(xm)
                # weight streams: wd on ACT (needed ~12us), wu on SP after x
                for gi in range(NG):
                    nc.scalar.dma_start(out=t_wd[:, gi * ne:(gi + 1) * ne],
                                        in_=wd.ap()[:, gi * ne:(gi + 1) * ne])
                for gi in range(NG):
                    nc.sync.dma_start(out=t_wu[:, gi * ne:(gi + 1) * ne],
                                      in_=wu.ap()[:, gi * ne:(gi + 1) * ne])

                # ---- conv gates: pr^T[16, T] = sum_k Wc_k^T @ E^T[:, k:k+T]
                p_cv = pconv.tile([N_PROC, T], F32)
                for k in range(5):
                    nc.tensor.matmul(p_cv, t_wc[:, k, :], t_et[:, k:k + T],
                                     start=(k == 0), stop=(k == 4))
                g_final = consts.tile([N_PROC, T], F32)
                nc.scalar.activation(out=g_final, in_=p_cv,
                                     func=mybir.ActivationFunctionType.Sigmoid,
                                     bias=t_cb, scale=1.0)
                nc.sync.dma_start(out=g.ap(), in_=g_final)
                g_bf = consts.tile([N_PROC, T], BF16)
                nc.scalar.copy(g_bf, g_final)
                nc.scalar.dma_start(out=d_g, in_=g_bf)

                # ---- X^T via PE transpose, per k-chunk collect 4 m-blocks
                for k in range(KH):
                    p_t = ptr.tile([128, TM, 128], F32, tag="pt")
                    for m in range(TM):
                        nc.tensor.transpose(p_t[:, m, :],
                                            t_x[m][:, k * 128:(k + 1) * 128], ident)
                    nc.vector.tensor_copy(t_xt[:, k, :],
                                          p_t.rearrange("p m t -> p (m t)"))

            with tc.tile_pool(name="dsT", bufs=1) as dspool, \
                 tc.tile_pool(name="gbc", bufs=1) as gbcp, \
                 tc.tile_pool(name="ysb", bufs=3) as ysb, \
                 tc.tile_pool(name="pd", bufs=4, space="PSUM") as pd, \
                 tc.tile_pool(name="py", bufs=3, space="PSUM") as py:

                t_ds = dspool.tile([128, N_PROC, T], BF16)    # Ds^T [r, n, t]
                # one fused broadcast DMA: every gate row to all 128 partitions
                gbig = gbcp.tile([128, N_PROC, T], BF16)
                nc.scalar.dma_start(out=gbig, in_=d_g.partition_broadcast(128))

                # ---- down matmuls + gate scale per expert
                for n in range(N_PROC):
                    p_d = pd.tile([128, T], F32, tag="pd")
                    for k in range(KH):
                        nc.tensor.matmul(p_d, t_wd[:, n, k, :], t_xt[:, k, :],
                                         start=(k == 0), stop=(k == KH - 1))
                    nc.vector.tensor_mul(t_ds[:, n, :], p_d, gbig[:, n, :])

                # ---- up matmuls: Y[m-tile, h-half] = sum_n Ds^T[n]^T @ Wu[n]
                for m in range(TM):
                    for h2 in range(2):
                        p_y = py.tile([128, 512], F32, tag="py")
                        for n in range(N_PROC):
                            nc.tensor.matmul(
                                p_y,
                                t_ds[:, n, m * 128:(m + 1) * 128],
                                t_wu[:, n, h2 * 512:(h2 + 1) * 512],
                                start=(n == 0), stop=(n == N_PROC - 1))
                        ys = ysb.tile([128, 512], F32, tag="ys")
                        nc.vector.tensor_copy(ys, p_y)
                        eng = nc.scalar if (m < 2) else nc.sync
                        eng.dma_start(
                            out=y.ap()[m * 128:(m + 1) * 128,
                                       h2 * 512:(h2 + 1) * 512],
                            in_=ys)
    _split_multiwait_drains(nc)
    return nc


_NC_CACHE = None


def _get_nc():
    global _NC_CACHE
    if _NC_CACHE is None:
        _NC_CACHE = _build()
    return _NC_CACHE


def _prep_in_maps(intermediate, enriched_activations, conv_w, conv_b,
                  down_proj, up_proj):
    X2 = np.ascontiguousarray(intermediate.reshape(B * S, H), dtype=np.float32)
    E2 = np.asarray(enriched_activations.reshape(B * S, N_IN), dtype=np.float32)
    wc_h = np.ascontiguousarray(np.asarray(conv_w, np.float32)[:, 0].transpose(1, 2, 0))
    cb_h = np.ascontiguousarray(np.asarray(conv_b, np.float32).reshape(N_PROC, 1))
    # pack weights into SBUF partition layout, bf16:
    # wd[p, n, k, r] = down_proj[n, k*128+p, r]
    wd_f = np.asarray(down_proj, np.float32).reshape(N_PROC, KH, 128, R)
    wd_h = np.ascontiguousarray(wd_f.transpose(2, 0, 1, 3).astype(ml_dtypes.bfloat16))
    # wu[p, n, h] = up_proj[n, p, h]
    wu_h = np.ascontiguousarray(
        np.asarray(up_proj, np.float32).transpose(1, 0, 2).astype(ml_dtypes.bfloat16))
    in_maps = []
    for c in range(N_CORES):
        s0 = c * T
        b = s0 // S
        ep = np.zeros((ET_W, N_IN), np.float32)
        lo, hi = s0 - 2, s0 + T + 2
        vlo, vhi = max(lo, b * S), min(hi, (b + 1) * S)
        ep[vlo - lo:vhi - lo] = E2[vlo:vhi]
        in_maps.append({
            "x": X2[s0:s0 + T].copy(),
            "et": np.ascontiguousarray(ep.T),
            "wc": wc_h, "cb": cb_h, "wd": wd_h, "wu": wu_h,
        })
    return in_maps


def run(inputs, trace=False, trace_kwargs=None):
    nc = _get_nc()
    in_maps = _prep_in_maps(**inputs)
    res = bass_utils.run_bass_kernel_spmd(
        nc, in_maps, core_ids=list(range(N_CORES)), trace=trace,
        **(trace_kwargs or {}))
    ys = np.concatenate([res.results[c]["y"] for c in range(N_CORES)], axis=0)
    gs = np.concatenate([res.results[c]["g"].T for c in range(N_CORES)], axis=0)
    out = ys.reshape(B, S, H).astype(np.float32)
    gates = gs.reshape(B, S, N_PROC).astype(np.float32)
    return (out, gates), res


def kernel(**inputs):
    outs, _ = run(inputs, trace=False)
    return outs
